# revision 2
# baseline (speedup 1.0000x reference)
"""Trainium2 Bass kernel for nn_AttentionHead_46660524703758.

Reference computation (per batch b of 8):
    keys   = x @ kw            [2048, 64]
    values = x @ vw            [2048, 64]
    scores = keys @ values.T / 8          (masked: keep col >= row)
    out    = softmax(scores, -1) @ values

Sharding: data-parallel over the batch dim, one batch per NeuronCore (8 cores).

Single software-pipelined pass (all matmuls fp16 in / fp32 PSUM):
    x tiles --SWDGE cast DMA (issued first)--> fp16
    x.T blocks via regular matmul vs identity; DVE drains PSUM->xT fp16
    proj: [kw|vw] stationary over xT seg -> [K^T; V^T]; SBUF DMAs replicate
        V^T to parts 0:64 / K^T to parts 64:128 for 2-tile row-group packing
    S^T pair (a,b): two concurrent K=64 matmuls (rows 0:63 / 64:127)
        -> ACT exp(s/8 - 6) -> P^T fp16; gpsimd masks the diagonal block
    AV: [V|1].T @ P^T accumulates into 4 persistent PSUM banks (one per
        512-col k-segment) DURING the loop, interleaved between S^T chunks
    epilogue per k-segment: drain O^T, PE-transpose, divide by colsum, DMA out

PE program order interleaves segment s+1 transposes with segment s
proj/S^T/AV so the engine never idles past the HAM throttle window.
"""
import sys

if "/opt/trn_rl_repo" not in sys.path:
    sys.path.insert(0, "/opt/trn_rl_repo")

import numpy as np

import concourse.bacc as bacc
import concourse.mybir as mybir
from concourse import tile
from concourse.bass_utils import run_bass_kernel_spmd
from concourse.masks import make_identity

B, N, H, E = 8, 2048, 1024, 64
P = 128
NT = N // P   # 16 n-tiles
HC = H // P   # 8 h-chunks
DT = mybir.dt.float16
F32 = mybir.dt.float32
EXP_BIAS = -6.0
EXP_SCALE = 0.125  # 1/sqrt(E)
N_WARM = 8

_cached_nc = None


def build_nc():
    nc = bacc.Bacc("TRN2", target_bir_lowering=False, debug=False, num_devices=8)
    x = nc.dram_tensor("x", [N, H], F32, kind="ExternalInput")
    kwt = nc.dram_tensor("kw", [H, E], F32, kind="ExternalInput")
    vwt = nc.dram_tensor("vw", [H, E], F32, kind="ExternalInput")
    out = nc.dram_tensor("out", [N, E], F32, kind="ExternalOutput")

    Exp = mybir.ActivationFunctionType.Exp

    with tile.TileContext(nc) as tc:
        with (
            tc.tile_pool(name="const", bufs=1) as const,
            tc.tile_pool(name="big", bufs=1) as big,
            tc.tile_pool(name="xin", bufs=16) as xin,
            tc.tile_pool(name="opool", bufs=2) as opool,
            tc.tile_pool(name="ps", bufs=4, space="PSUM") as psp,
            tc.tile_pool(name="av", bufs=4, space="PSUM") as avp,
        ):
            # --- minimal gpsimd constants, then x DMAs immediately ---
            ident_h = const.tile([P, P], DT)
            make_identity(nc, ident_h[:])
            warm_src = const.tile([P, 512], DT)
            nc.gpsimd.memset(warm_src[:], 0.001)
            bias_t = const.tile([P, 1], F32)
            nc.gpsimd.memset(bias_t[:], EXP_BIAS)
            vnat = big.tile([P, NT * (E + 1)], DT)
            vnat3 = vnat[:].rearrange("p (j c) -> p j c", c=E + 1)
            nc.gpsimd.memset(vnat3[:, :, E : E + 1], 1.0)

            # x input DMAs (SWDGE cast fp32->fp16), descriptor gen starts now
            xts = []
            for t in range(NT):
                xt = xin.tile([P, H], DT, tag="xt", name=f"xt{t}")
                nc.gpsimd.dma_start(xt[:], x.ap()[t * P : (t + 1) * P, :])
                xts.append(xt)

            # --- weights via sync HWDGE + DVE pack to fp16 [K|V] ---
            kw_sb = const.tile([P, HC * E], F32)
            vw_sb = const.tile([P, HC * E], F32)
            nc.sync.dma_start(
                kw_sb[:].rearrange("p (c e) -> p c e", c=HC),
                kwt.ap().rearrange("(c p) e -> p c e", p=P),
            )
            nc.sync.dma_start(
                vw_sb[:].rearrange("p (c e) -> p c e", c=HC),
                vwt.ap().rearrange("(c p) e -> p c e", p=P),
            )
            kvw = const.tile([P, HC, P], DT)
            nc.vector.tensor_copy(
                kvw[:, :, 0:E], kw_sb[:].rearrange("p (c e) -> p c e", c=HC)
            )
            nc.vector.tensor_copy(
                kvw[:, :, E:P], vw_sb[:].rearrange("p (c e) -> p c e", c=HC)
            )

            # --- main tensors ---
            xT = big.tile([P, HC * N], DT)  # [h_in_chunk, c*2048 + n]
            xT3 = xT[:].rearrange("p (c n) -> p c n", c=HC)
            kvT = big.tile([P, N], DT)      # rows 0:64 K^T, 64:128 V^T
            vT_lo = big.tile([64, N], DT)   # V^T copy on parts 0:64
            kT_hi = big.tile([P, N], DT)    # rows 64:128 hold a K^T copy
            pT = big.tile([P, NT * N], DT)  # [v_in_tile, i*2048 + k]
            out_sb = big.tile([P, NT * E], F32)

            # --- PE warm-up: trip the HAM clock-gate while DMA flows ---
            for w in range(N_WARM):
                wp = psp.tile([P, 512], F32, tag="ps", name=f"warm{w}")
                nc.tensor.matmul(wp[:], ident_h[:], warm_src[:], start=True, stop=True)

            # --- persistent AV accumulators, one PSUM bank per k-segment ---
            av_tiles = [
                avp.tile([P, 512], F32, tag="av", name=f"av{c}") for c in range(4)
            ]

            # ---------- emission helpers ----------
            def emit_tr(t):
                for half in range(2):
                    trp = psp.tile([P, 512], F32, tag="ps", name=f"tr{t}_{half}")
                    for cc in range(4):
                        c = half * 4 + cc
                        nc.tensor.matmul(
                            trp[:, cc * P : (cc + 1) * P],
                            xts[t][:, c * P : (c + 1) * P],
                            ident_h[:],
                            start=True,
                            stop=True,
                        )
                    nc.vector.tensor_copy(
                        xT3[:, half * 4 : half * 4 + 4, t * P : (t + 1) * P],
                        trp[:].rearrange("p (c n) -> p c n", c=4),
                    )

            def emit_proj(s):
                ps_ = psp.tile([P, 512], F32, tag="ps", name=f"proj{s}")
                for c in range(HC):
                    nc.tensor.matmul(
                        ps_[:],
                        kvw[:, c, :],
                        xT3[:, c, s * 512 : (s + 1) * 512],
                        start=(c == 0),
                        stop=(c == HC - 1),
                    )
                nc.vector.tensor_copy(kvT[:, s * 512 : (s + 1) * 512], ps_[:])
                nc.sync.dma_start(
                    vT_lo[:, s * 512 : (s + 1) * 512],
                    kvT[64:128, s * 512 : (s + 1) * 512],
                )
                nc.sync.dma_start(
                    kT_hi[64:128, s * 512 : (s + 1) * 512],
                    kvT[0:64, s * 512 : (s + 1) * 512],
                )

            def emit_vtr(s):
                vtr = psp.tile([P, 4 * E], F32, tag="ps", name=f"vtr{s}")
                for jj in range(4):
                    j = 4 * s + jj
                    nc.tensor.matmul(
                        vtr[:, jj * E : (jj + 1) * E],
                        vT_lo[:, j * P : (j + 1) * P],
                        ident_h[0:64, 0:64],
                        start=True,
                        stop=True,
                    )
                nc.vector.tensor_copy(
                    vnat3[:, 4 * s : 4 * s + 4, 0:E],
                    vtr[:].rearrange("p (j e) -> p j e", j=4),
                )

            def emit_affine(i):
                dg = i * N + i * P
                nc.gpsimd.affine_select(
                    out=pT[:, dg : dg + P],
                    in_=pT[:, dg : dg + P],
                    compare_op=mybir.AluOpType.is_ge,
                    fill=0.0,
                    base=0,
                    pattern=[[-1, P]],
                    channel_multiplier=1,
                )

            def emit_av(i, c):
                nn = min(512, (i + 1) * P - c * 512)
                nc.tensor.matmul(
                    av_tiles[c][0 : E + 1, 0:nn],
                    vnat3[:, i, :],
                    pT[:, i * N + c * 512 : i * N + c * 512 + nn],
                    start=(i == 4 * c),
                    stop=(i == NT - 1),
                )

            av_queue = []  # pending (i, c) AV matmuls

            def drain_av(k):
                for _ in range(min(k, len(av_queue))):
                    i, c = av_queue.pop(0)
                    emit_av(i, c)

            def emit_st_pair(p, interleave_av):
                a, b = 2 * p, 2 * p + 1
                kend = (b + 1) * P
                for sc in range((kend + 511) // 512):
                    ks = sc * 512
                    nn_a = min(512, max(0, (a + 1) * P - ks))
                    nn_b = min(512, kend - ks)
                    st_a = None
                    if nn_a > 0:
                        st_a = psp.tile([P, 512], F32, tag="ps", name=f"sta{p}_{sc}")
                        nc.tensor.matmul(
                            st_a[:, 0:nn_a],
                            vT_lo[:, a * P : (a + 1) * P],
                            kvT[0:64, ks : ks + nn_a],
                            start=True,
                            stop=True,
                        )
                    st_b = psp.tile([P, 512], F32, tag="ps", name=f"stb{p}_{sc}")
                    nc.tensor.matmul(
                        st_b[:, 0:nn_b],
                        kvT[64:128, b * P : (b + 1) * P],
                        kT_hi[64:128, ks : ks + nn_b],
                        start=True,
                        stop=True,
                    )
                    if st_a is not None:
                        nc.scalar.activation(
                            pT[:, a * N + ks : a * N + ks + nn_a],
                            st_a[:, 0:nn_a],
                            Exp,
                            bias=bias_t[:],
                            scale=EXP_SCALE,
                        )
                    nc.scalar.activation(
                        pT[:, b * N + ks : b * N + ks + nn_b],
                        st_b[:, 0:nn_b],
                        Exp,
                        bias=bias_t[:],
                        scale=EXP_SCALE,
                    )
                    if sc == a // 4:
                        emit_affine(a)
                    if sc == b // 4:
                        emit_affine(b)
                    if interleave_av:
                        drain_av(2)
                av_queue.extend((a, c) for c in range((a + 4) // 4))
                av_queue.extend((b, c) for c in range((b + 4) // 4))

            # ---------- software-pipelined main loop ----------
            for t in range(4):
                emit_tr(t)
            emit_proj(0)
            emit_vtr(0)
            for s in range(1, 4):
                emit_tr(4 * s)
                emit_tr(4 * s + 1)
                emit_st_pair(2 * (s - 1), interleave_av=(s >= 2))
                emit_tr(4 * s + 2)
                emit_tr(4 * s + 3)
                emit_st_pair(2 * (s - 1) + 1, interleave_av=(s >= 2))
                emit_proj(s)
                emit_vtr(s)
            emit_st_pair(6, interleave_av=True)
            emit_st_pair(7, interleave_av=True)
            drain_av(len(av_queue))

            # ---------- epilogue: per k-segment drain, transpose, divide ----------
            for c in range(4):
                oT_c = opool.tile([E + 1, 512], DT, tag="oT", name=f"oT{c}")
                nc.vector.tensor_copy(oT_c[:], av_tiles[c][0 : E + 1, :])
                for kt in range(4):
                    tr = psp.tile([P, E + 1], F32, tag="ps", name=f"otr{c}_{kt}")
                    nc.tensor.matmul(
                        tr[:, 0 : E + 1],
                        oT_c[:, kt * P : (kt + 1) * P],
                        ident_h[0 : E + 1, 0 : E + 1],
                        start=True,
                        stop=True,
                    )
                    kti = 4 * c + kt
                    rec = opool.tile([P, 1], F32, tag="rec", name=f"rec{kti}")
                    nc.vector.reciprocal(rec[:], tr[:, E : E + 1])
                    nc.vector.tensor_scalar_mul(
                        out_sb[:, kti * E : (kti + 1) * E], tr[:, 0:E], rec[:]
                    )
                nc.sync.dma_start(
                    out.ap().rearrange("(t p) e -> p t e", p=P)[:, 4 * c : 4 * c + 4, :],
                    out_sb[:].rearrange("p (t e) -> p t e", t=NT)[
                        :, 4 * c : 4 * c + 4, :
                    ],
                )

    nc.finalize()
    return nc


def _get_nc():
    global _cached_nc
    if _cached_nc is None:
        _cached_nc = build_nc()
    return _cached_nc


def kernel(input, k, q, v, **extra_bass_kwargs):
    """Full-input entry point: shards batch across 8 cores, gathers output."""
    del q  # reference computes queries but never uses them
    input = np.ascontiguousarray(np.asarray(input, dtype=np.float32))
    k = np.ascontiguousarray(np.asarray(k, dtype=np.float32))
    v = np.ascontiguousarray(np.asarray(v, dtype=np.float32))
    nc = _get_nc()
    in_maps = [{"x": input[b], "kw": k, "vw": v} for b in range(B)]
    res = run_bass_kernel_spmd(
        nc, in_maps, core_ids=list(range(B)), **extra_bass_kwargs
    )
    out = np.stack([r["out"] for r in res.results]).astype(np.float32)
    if extra_bass_kwargs:
        kernel.last_results = res
    return out


# revision 3
# speedup vs baseline: 1.1829x; 1.1829x over previous
"""Trainium2 Bass kernel for nn_AttentionHead_46660524703758.

Reference computation (per batch b of 8):
    keys   = x @ kw            [2048, 64]
    values = x @ vw            [2048, 64]
    scores = keys @ values.T / 8          (masked: keep col >= row)
    out    = softmax(scores, -1) @ values

Sharding: data-parallel over the batch dim, one batch per NeuronCore (8 cores).

Single software-pipelined pass over 8 two-tile units (all matmuls fp16 in /
fp32 PSUM):
    x 2-tile units --SWDGE cast DMA (issued first)--> fp16
    x.T blocks via regular matmul vs identity; DVE drains PSUM->xT fp16
    per-unit proj: [kw|vw] stationary, N=256 -> [K^T; V^T]; two parallel
        HWDGE queues replicate V^T to parts 0:64 / K^T to parts 64:128
    S^T pair (2u, 2u+1): two concurrent K=64 matmuls (rows 0:63 / 64:127)
        -> ACT exp(s/8 - 6) -> P^T fp16; gpsimd masks the diagonal block
    AV: [V|1].T @ P^T accumulates into 4 persistent PSUM banks (one per
        512-col k-segment) DURING the loop, drained into ST/TR slack
    epilogue per k-segment: drain O^T fp16, PE-transpose, reciprocal on DVE,
        scale on ScalarE (per-partition AP scale), DMA out

PE program order interleaves unit u+1 transposes with unit u proj/S^T/AV;
filler matmuls bridge the known DMA-paced stalls to keep the HAM clock-gate
at K=8/8.
"""
import sys

if "/opt/trn_rl_repo" not in sys.path:
    sys.path.insert(0, "/opt/trn_rl_repo")

import numpy as np

import concourse.bacc as bacc
import concourse.mybir as mybir
from concourse import tile
from concourse.bass_utils import run_bass_kernel_spmd
from concourse.masks import make_identity

B, N, H, E = 8, 2048, 1024, 64
P = 128
NT = N // P   # 16 n-tiles
NU = 8        # two-tile units
HC = H // P   # 8 h-chunks
DT = mybir.dt.float16
F32 = mybir.dt.float32
EXP_BIAS = -6.0
EXP_SCALE = 0.125  # 1/sqrt(E)
N_WARM = 12

_cached_nc = None


def build_nc():
    nc = bacc.Bacc("TRN2", target_bir_lowering=False, debug=False, num_devices=8)
    x = nc.dram_tensor("x", [N, H], F32, kind="ExternalInput")
    kwt = nc.dram_tensor("kw", [H, E], F32, kind="ExternalInput")
    vwt = nc.dram_tensor("vw", [H, E], F32, kind="ExternalInput")
    out = nc.dram_tensor("out", [N, E], F32, kind="ExternalOutput")

    Exp = mybir.ActivationFunctionType.Exp
    Copy = mybir.ActivationFunctionType.Copy

    with tile.TileContext(nc) as tc:
        with (
            tc.tile_pool(name="const", bufs=1) as const,
            tc.tile_pool(name="big", bufs=1) as big,
            tc.tile_pool(name="xin", bufs=8) as xin,
            tc.tile_pool(name="opool", bufs=2) as opool,
            tc.tile_pool(name="ps", bufs=4, space="PSUM") as psp,
            tc.tile_pool(name="av", bufs=4, space="PSUM") as avp,
        ):
            # --- minimal gpsimd constants, then x DMAs immediately ---
            ident_h = const.tile([P, P], DT)
            make_identity(nc, ident_h[:])
            warm_src = const.tile([P, 512], DT)
            nc.gpsimd.memset(warm_src[:], 0.001)
            bias_t = const.tile([P, 1], F32)
            nc.gpsimd.memset(bias_t[:], EXP_BIAS)
            vnat = big.tile([P, NT * (E + 1)], DT)
            vnat3 = vnat[:].rearrange("p (j c) -> p j c", c=E + 1)
            nc.gpsimd.memset(vnat3[:, :, E : E + 1], 1.0)

            # x input DMAs: 8 two-tile units (SWDGE cast fp32->fp16)
            xus = []
            for u in range(NU):
                xu = xin.tile([P, 2 * H], DT, tag="xu", name=f"xu{u}")
                nc.gpsimd.dma_start(
                    xu[:].rearrange("p (q h) -> p q h", q=2),
                    x.ap()[u * 2 * P : (u + 1) * 2 * P, :].rearrange(
                        "(q p) h -> p q h", p=P
                    ),
                )
                xus.append(xu)

            # --- weights via sync HWDGE + DVE pack to fp16 [K|V] ---
            kw_sb = const.tile([P, HC * E], F32)
            vw_sb = const.tile([P, HC * E], F32)
            nc.sync.dma_start(
                kw_sb[:].rearrange("p (c e) -> p c e", c=HC),
                kwt.ap().rearrange("(c p) e -> p c e", p=P),
            )
            nc.sync.dma_start(
                vw_sb[:].rearrange("p (c e) -> p c e", c=HC),
                vwt.ap().rearrange("(c p) e -> p c e", p=P),
            )
            kvw = const.tile([P, HC, P], DT)
            nc.vector.tensor_copy(
                kvw[:, :, 0:E], kw_sb[:].rearrange("p (c e) -> p c e", c=HC)
            )
            nc.vector.tensor_copy(
                kvw[:, :, E:P], vw_sb[:].rearrange("p (c e) -> p c e", c=HC)
            )

            # --- main tensors ---
            xT = big.tile([P, HC * N], DT)  # [h_in_chunk, c*2048 + n]
            xT3 = xT[:].rearrange("p (c n) -> p c n", c=HC)
            kvT = big.tile([P, N], DT)      # rows 0:64 K^T, 64:128 V^T
            vT_lo = big.tile([64, N], DT)   # V^T copy on parts 0:64
            kT_hi = big.tile([P, N], DT)    # rows 64:128 hold a K^T copy
            pT = big.tile([P, NT * N], DT)  # [v_in_tile, i*2048 + k]
            out_sb = big.tile([P, NT * E], F32)

            # --- PE warm-up: trip the HAM clock-gate while DMA flows ---
            for w in range(N_WARM):
                wp = psp.tile([P, 512], F32, tag="ps", name=f"warm{w}")
                nc.tensor.matmul(wp[:], ident_h[:], warm_src[:], start=True, stop=True)

            # --- persistent AV accumulators, one PSUM bank per k-segment ---
            av_tiles = [
                avp.tile([P, 512], F32, tag="av", name=f"av{c}") for c in range(4)
            ]

            fill_ctr = [0]
            av_queue = []  # pending (i, c) AV matmuls

            # ---------- emission helpers ----------
            def emit_tr(t):
                u, q = t // 2, t % 2
                for half in range(2):
                    trp = psp.tile([P, 512], F32, tag="ps", name=f"tr{t}_{half}")
                    for cc in range(4):
                        c = half * 4 + cc
                        nc.tensor.matmul(
                            trp[:, cc * P : (cc + 1) * P],
                            xus[u][:, q * H + c * P : q * H + (c + 1) * P],
                            ident_h[:],
                            start=True,
                            stop=True,
                        )
                    nc.vector.tensor_copy(
                        xT3[:, half * 4 : half * 4 + 4, t * P : (t + 1) * P],
                        trp[:].rearrange("p (c n) -> p c n", c=4),
                    )

            def emit_proju(u):
                c0 = u * 256
                ps_ = psp.tile([P, 256], F32, tag="ps", name=f"proj{u}")
                for c in range(HC):
                    nc.tensor.matmul(
                        ps_[:],
                        kvw[:, c, :],
                        xT3[:, c, c0 : c0 + 256],
                        start=(c == 0),
                        stop=(c == HC - 1),
                    )
                nc.vector.tensor_copy(kvT[:, c0 : c0 + 256], ps_[:])
                nc.sync.dma_start(
                    vT_lo[:, c0 : c0 + 256], kvT[64:128, c0 : c0 + 256]
                )
                nc.scalar.dma_start(
                    kT_hi[64:128, c0 : c0 + 256], kvT[0:64, c0 : c0 + 256]
                )

            def emit_vtru(u):
                vtr = psp.tile([P, 2 * E], F32, tag="ps", name=f"vtr{u}")
                for jj in range(2):
                    j = 2 * u + jj
                    nc.tensor.matmul(
                        vtr[:, jj * E : (jj + 1) * E],
                        vT_lo[:, j * P : (j + 1) * P],
                        ident_h[0:64, 0:64],
                        start=True,
                        stop=True,
                    )
                nc.vector.tensor_copy(
                    vnat3[:, 2 * u : 2 * u + 2, 0:E],
                    vtr[:].rearrange("p (j e) -> p j e", j=2),
                )

            def emit_affine(i):
                dg = i * N + i * P
                nc.gpsimd.affine_select(
                    out=pT[:, dg : dg + P],
                    in_=pT[:, dg : dg + P],
                    compare_op=mybir.AluOpType.is_ge,
                    fill=0.0,
                    base=0,
                    pattern=[[-1, P]],
                    channel_multiplier=1,
                )

            def emit_av(i, c):
                nn = min(512, (i + 1) * P - c * 512)
                nc.tensor.matmul(
                    av_tiles[c][0 : E + 1, 0:nn],
                    vnat3[:, i, :],
                    pT[:, i * N + c * 512 : i * N + c * 512 + nn],
                    start=(i == 4 * c),
                    stop=(i == NT - 1),
                )

            def emit_filler(n):
                for _ in range(n):
                    wp = psp.tile(
                        [P, 512], F32, tag="ps", name=f"fill{fill_ctr[0]}"
                    )
                    fill_ctr[0] += 1
                    nc.tensor.matmul(
                        wp[:], ident_h[:], warm_src[:], start=True, stop=True
                    )

            def pe_slack(n, fill=False):
                k = min(n, len(av_queue))
                for _ in range(k):
                    i, c = av_queue.pop(0)
                    emit_av(i, c)
                if fill and n > k:
                    emit_filler(n - k)

            def emit_st_pair(p):
                a, b = 2 * p, 2 * p + 1
                kend = (b + 1) * P
                for sc in range((kend + 511) // 512):
                    ks = sc * 512
                    nn_a = min(512, max(0, (a + 1) * P - ks))
                    nn_b = min(512, kend - ks)
                    st_a = None
                    if nn_a > 0:
                        st_a = psp.tile([P, 512], F32, tag="ps", name=f"sta{p}_{sc}")
                        nc.tensor.matmul(
                            st_a[:, 0:nn_a],
                            vT_lo[:, a * P : (a + 1) * P],
                            kvT[0:64, ks : ks + nn_a],
                            start=True,
                            stop=True,
                        )
                    st_b = psp.tile([P, 512], F32, tag="ps", name=f"stb{p}_{sc}")
                    nc.tensor.matmul(
                        st_b[:, 0:nn_b],
                        kvT[64:128, b * P : (b + 1) * P],
                        kT_hi[64:128, ks : ks + nn_b],
                        start=True,
                        stop=True,
                    )
                    if st_a is not None:
                        nc.scalar.activation(
                            pT[:, a * N + ks : a * N + ks + nn_a],
                            st_a[:, 0:nn_a],
                            Exp,
                            bias=bias_t[:],
                            scale=EXP_SCALE,
                        )
                    nc.scalar.activation(
                        pT[:, b * N + ks : b * N + ks + nn_b],
                        st_b[:, 0:nn_b],
                        Exp,
                        bias=bias_t[:],
                        scale=EXP_SCALE,
                    )
                    if sc == a // 4:
                        emit_affine(a)
                    if sc == b // 4:
                        emit_affine(b)
                    pe_slack(2)
                av_queue.extend((a, c) for c in range((a + 4) // 4))
                av_queue.extend((b, c) for c in range((b + 4) // 4))

            def emit_epilogue(c):
                oT_c = opool.tile([E + 1, 512], DT, tag="oT", name=f"oT{c}")
                nc.vector.tensor_copy(oT_c[:], av_tiles[c][0 : E + 1, :])
                tr4 = psp.tile([P, 4 * (E + 1)], F32, tag="ps", name=f"otr{c}")
                tr4v = tr4[:].rearrange("p (k c) -> p k c", k=4)
                for kt in range(4):
                    nc.tensor.matmul(
                        tr4[:, kt * (E + 1) : (kt + 1) * (E + 1)],
                        oT_c[:, kt * P : (kt + 1) * P],
                        ident_h[0 : E + 1, 0 : E + 1],
                        start=True,
                        stop=True,
                    )
                rec4 = opool.tile([P, 4], F32, tag="rec", name=f"rec{c}")
                nc.vector.reciprocal(
                    rec4[:].rearrange("p (k o) -> p k o", o=1),
                    tr4v[:, :, E : E + 1],
                )
                for kt in range(4):
                    kti = 4 * c + kt
                    nc.scalar.activation(
                        out_sb[:, kti * E : (kti + 1) * E],
                        tr4v[:, kt, 0:E],
                        Copy,
                        scale=rec4[:, kt : kt + 1],
                    )
                nc.sync.dma_start(
                    out.ap().rearrange("(t p) e -> p t e", p=P)[:, 4 * c : 4 * c + 4, :],
                    out_sb[:].rearrange("p (t e) -> p t e", t=NT)[
                        :, 4 * c : 4 * c + 4, :
                    ],
                )

            # ---------- software-pipelined main loop over units ----------
            emit_tr(0)
            emit_tr(1)
            emit_proju(0)
            pe_slack(4, fill=True)
            for u in range(1, NU):
                emit_tr(2 * u)
                emit_tr(2 * u + 1)
                pe_slack(2, fill=(u <= 4))
                emit_st_pair(u - 1)
                emit_vtru(u - 1)
                emit_proju(u)
            emit_st_pair(NU - 1)
            emit_vtru(NU - 1)

            # ---------- final AV drain + per-segment epilogue ----------
            rest = av_queue[:]
            av_queue.clear()
            for c in range(4):
                for i, cc in rest:
                    if cc == c:
                        emit_av(i, cc)
                emit_epilogue(c)

    nc.finalize()
    return nc


def _get_nc():
    global _cached_nc
    if _cached_nc is None:
        _cached_nc = build_nc()
    return _cached_nc


def kernel(input, k, q, v, **extra_bass_kwargs):
    """Full-input entry point: shards batch across 8 cores, gathers output."""
    del q  # reference computes queries but never uses them
    input = np.ascontiguousarray(np.asarray(input, dtype=np.float32))
    k = np.ascontiguousarray(np.asarray(k, dtype=np.float32))
    v = np.ascontiguousarray(np.asarray(v, dtype=np.float32))
    nc = _get_nc()
    in_maps = [{"x": input[b], "kw": k, "vw": v} for b in range(B)]
    res = run_bass_kernel_spmd(
        nc, in_maps, core_ids=list(range(B)), **extra_bass_kwargs
    )
    out = np.stack([r["out"] for r in res.results]).astype(np.float32)
    if extra_bass_kwargs:
        kernel.last_results = res
    return out
